# revision 41
# baseline (speedup 1.0000x reference)
"""AdaPool3d Trainium2 kernel — 8-core data parallel, v2.

x [4,64,16,112,112] f32, beta [8,56,56] f32 -> out [4,64,8,56,56] f32.
256 (b,c) images sharded 32/core, processed as 16 pairs/core.

Per image [16,112,112], SBUF layout X [128,1568] bf16 (DMA-cast of
SIG*x): partition p=(kd,od,oh3), free f=(kh,kw,ohp,ow); d=2*od+kd,
h=16*ohp+2*oh3+kh, w=2*ow+kw.  All window reductions are PE matmuls
against one constant lhsT (0.125 kd-pair selector, kd-dup for the avg,
col-halves 0:64/64:128 for even/odd image of a pair so the four
e/ex/f/fx sums of a pair share PSUM banks as [128,392]).

Math: u = x/avg via RECIP1 (NOT-trick+1NR) on the small [128,392] avg;
DS = DICEU(X, rA) = u*nb*(C1 - (1+u^2)*nb), nb=NOT(1+u^2) — an 8-op
fused DVE op whose NR seed scale is absorbed into C1=-8.5, making
DS = dsc/2 * G with G=18.0329.  The host pre-scales x by SIG=G/2 so a
single ACT exp over [X|DS] with scale=1/SIG yields [e^x | e^dsc].
em/edscw divisions fold beta and all scales into two host constants
invBE = SIG/(1-beta), invBF = SIG/beta; out = pM/(pE*invBE) +
pX/(pF*invBF) via two DIV1 ops and one bf16 add per pair.
"""

import numpy as np

_NCORES = 8
_IMGS = 32          # images per core
_PAIRS = 16
_OD, _OH, _OW = 8, 56, 56
_OHP, _OH3 = 7, 8   # oh = 8*ohp + oh3
_FD = 1568          # per-partition free elems = 4*392
_NW = 392           # windows per q-group = 7*56

_G = 18.032925      # DICEU gain (C1 = -8.5)
_SIG = _G / 2.0
_C1D = -8.5
_R0, _R1 = -0.23549792, 2.0017324  # NOT-trick + 1NR recip constants

_cache = {}


def _register_op(name, spec):
    from concourse.dve_spec import lower, _has_src1
    from concourse import dve_ops
    from concourse.dve_uop import DveOpSpec

    for op in dve_ops.OPS:
        if op.name == name:
            return op
    row = dve_ops._CUSTOM_DVE_ROW_BASE + len(dve_ops.OPS)
    assert row < 0x20
    dve_ops._SUB_OPCODE_FOR_NAME[name] = row
    shas = {}
    for ver in ("v3", "v4"):
        try:
            uops = lower(spec, ver=ver)
            shas[ver] = DveOpSpec(
                name=name, opcode=row, uops=uops, rd1_en=_has_src1(spec)
            ).sha(ver)
        except Exception:
            pass
    op = dve_ops.DveOp(name, spec, subdim=False, uops_sha=shas)
    dve_ops.OPS.append(op)
    dve_ops.CUSTOM_DVE_SPECS[name] = spec
    return op


def _np_not(z):
    return (~np.asarray(z, np.float32).view(np.int32)).view(np.float32)


def _np_r1(z, s0, s1):
    y0 = _np_not(z) * np.float32(s0)
    return y0 * (np.float32(s1) - z * y0)


def _register_custom_ops():
    """RECIP1_EPS: approx(1/(x+eps)), 1-NR; DICEU: fused u=x*r,
    dsc-like = u*nb*(C1-(1+u^2)*nb); DIV1: num*approx(1/den), 1-NR."""
    from concourse.dve_spec import Spec, Src0, Src1, Bin, AluOp, sq, One, C0, C1, C2

    _ze = Src0 + C2
    _nbe = Bin(AluOp.BITWISE_NOT, _ze, _ze)
    _y0e = _nbe * C0
    _y1e = _y0e * (C1 - _ze * _y0e)
    recip1 = _register_op(
        "RECIP1_EPS_ANT",
        Spec(
            body=_y1e,
            reference=lambda in0, in1, s0, s1, imm2: _np_r1(
                in0.astype(np.float32) + np.float32(imm2), s0, s1
            ),
        ),
    )

    _u = Src0 * Src1
    _z = sq(_u) + One
    _nb = Bin(AluOp.BITWISE_NOT, _z, _z)
    _q = _z * _nb
    _w = C1 - _q
    _m = _u * _nb

    def _diceu_ref(in0, in1, s0, s1, imm2):
        u = in0.astype(np.float32) * in1.astype(np.float32)
        z = (1.0 + u * u).astype(np.float32)
        nb = _np_not(z)
        return (u * nb).astype(np.float32) * (
            np.float32(s1) - (z * nb).astype(np.float32)
        )

    diceu = _register_op("DICEU_ANT", Spec(body=_m * _w, reference=_diceu_ref))

    _nbd = Bin(AluOp.BITWISE_NOT, Src1, Src1)
    _y0d = _nbd * C0
    _y1d = _y0d * (C1 - Src1 * _y0d)
    div1 = _register_op(
        "DIV1_ANT",
        Spec(
            body=Src0 * _y1d,
            reference=lambda in0, in1, s0, s1, imm2: in0
            * _np_r1(in1.astype(np.float32), s0, s1),
        ),
    )
    return recip1, diceu, div1


def _build():
    if "nc" in _cache:
        return _cache["nc"]
    import concourse.bass as bass
    import concourse.bacc as bacc
    import concourse.mybir as mybir
    from concourse.tile import TileContext
    from contextlib import ExitStack

    RECIP1, DICEU, DIV1 = _register_custom_ops()
    f32, bf16 = mybir.dt.float32, mybir.dt.bfloat16
    AF = mybir.ActivationFunctionType
    MUL, ADD = mybir.AluOpType.mult, mybir.AluOpType.add

    nc = bacc.Bacc(None, target_bir_lowering=False, debug=False)
    # host pre-rearranged pairs: x [pair, p=(kd,od,oh3), f=(kh,kw,img,ohp,ow)]
    x_d = nc.dram_tensor("x", [_PAIRS, 128, 2 * _FD], bf16, kind="ExternalInput")
    lhs_d = nc.dram_tensor("lhs", [128, 128], bf16, kind="ExternalInput")
    idn_d = nc.dram_tensor("idn", [128, 128], bf16, kind="ExternalInput")
    ibe_d = nc.dram_tensor("invbe", [128, _NW], f32, kind="ExternalInput")
    ibf_d = nc.dram_tensor("invbf", [128, _NW], f32, kind="ExternalInput")
    out_d = nc.dram_tensor("out", [_PAIRS, 128, _NW], bf16, kind="ExternalOutput")

    x_ap = x_d.ap()
    out_ap = out_d.ap()

    with TileContext(nc) as tc, ExitStack() as ctx:
        const = ctx.enter_context(tc.tile_pool(name="const", bufs=1))
        xin = ctx.enter_context(tc.tile_pool(name="xin", bufs=4))
        ef = ctx.enter_context(tc.tile_pool(name="ef", bufs=4))
        mx = ctx.enter_context(tc.tile_pool(name="mx", bufs=4))
        sm = ctx.enter_context(tc.tile_pool(name="sm", bufs=3))
        pss = ctx.enter_context(tc.tile_pool(name="pss", bufs=2, space="PSUM"))

        # consts on the gpsimd queue so pair-0's x DMA leads the sync queue
        lhs_t = const.tile([128, 128], bf16, name="lhsT")
        nc.gpsimd.dma_start(out=lhs_t[:], in_=lhs_d.ap())
        idn_t = const.tile([128, 128], bf16, name="idnT")
        nc.gpsimd.dma_start(out=idn_t[:], in_=idn_d.ap())
        ibef_t = const.tile([128, 2 * _NW], f32, name="ibef")
        nc.gpsimd.dma_start(out=ibef_t[:, 0:_NW], in_=ibe_d.ap())
        nc.gpsimd.dma_start(out=ibef_t[:, _NW:2 * _NW], in_=ibf_d.ap())

        def emit_combine(pp, pEF, pMX):
            # ---- combine (per pair, [128, 2*392] = 2 images x {em, ed})
            cEF = sm.tile([128, 2 * _NW], f32, tag="cEF", name="cEF")
            nc.vector.tensor_tensor(
                cEF[:], pEF[:, :, 0:_NW], ibef_t[:], op=MUL
            )
            emed = sm.tile([128, 2 * _NW], bf16, tag="emed", name="emed")
            nc.vector._custom_dve(
                DIV1, out=emed[:], in0=pMX[:, :, 0:_NW], in1=cEF[:],
                s0=_R0, s1=_R1,
            )
            # em + ed on the (idle) PE: two identity matmuls accumulating
            # into the pMX bank the DIV1 just freed; ACT evacuates w/ cast.
            po = pMX[:, 0, 0:_NW]
            nc.tensor.matmul(po, idn_t[:], emed[:, 0:_NW],
                             start=True, stop=False, skip_group_check=True)
            nc.tensor.matmul(po, idn_t[:], emed[:, _NW:2 * _NW],
                             start=False, stop=True, skip_group_check=True)
            oc = sm.tile([128, _NW], bf16, tag="oc", name="oc")
            nc.scalar.copy(oc[:], po)
            nc.gpsimd.dma_start(out=out_ap[pp], in_=oc[:])

        prev = None
        for p in range(_PAIRS):
            # ---- load pair; [Xpair | DSpair] tile, f=(kh,kw,img,ohp,ow).
            # 4 chunked DMAs (one khw block each) so the avg matmuls start
            # as soon as their block lands instead of after the full 1.6MB.
            XX = xin.tile([128, 4 * _FD], bf16, tag="XX")
            for k in range(4):
                nc.sync.dma_start(
                    out=XX[:, k * 2 * _NW:(k + 1) * 2 * _NW],
                    in_=x_ap[p][:, k * 2 * _NW:(k + 1) * 2 * _NW],
                )
            Xf = XX[:, 0:2 * _FD]

            # ---- PSUM: pEF holds avg (kd-dup) then E/F sums; pMX holds M/X
            pEF = pss.tile([128, 2, 512], f32, tag="pEF", name="pEF")
            pMX = pss.tile([128, 2, 512], f32, tag="pMX", name="pMX")
            for j in (0, 1):
                for k in range(4):
                    nc.tensor.matmul(
                        pEF[:, j, 0:_NW], lhs_t[:],
                        XX[:, k * 2 * _NW + j * _NW:k * 2 * _NW + (j + 1) * _NW],
                        start=(k == 0), stop=(k == 3),
                        skip_group_check=True,
                    )

            # ---- E = exp(x) early: depends only on the DMA, not on DICEU
            EF = ef.tile([128, 4 * _FD], bf16, tag="EF")
            nc.scalar.activation(EF[:, 0:2 * _FD], Xf, AF.Exp, scale=1.0 / _SIG)

            # ---- rA2 = approx 1/avg  (f32, [128, 784])
            rA2 = sm.tile([128, 2 * _NW], f32, tag="rA2")
            nc.vector._custom_dve(
                RECIP1, out=rA2[:], in0=pEF[:, :, 0:_NW],
                s0=_R0, s1=_R1, imm2=1e-12,
            )

            # ---- DS = DICEU(X, bcast rA2) into XX[:, 2FD:4FD]
            rA_b = (
                rA2[:]
                .rearrange("q (s n) -> q s n", s=1)
                .broadcast_to([128, 4, 2 * _NW])
            )
            nc.vector._custom_dve(
                DICEU, out=XX[:, 2 * _FD:4 * _FD], in0=Xf, in1=rA_b, s1=_C1D
            )

            # ---- F = exp(dsc)
            nc.scalar.activation(
                EF[:, 2 * _FD:4 * _FD], XX[:, 2 * _FD:4 * _FD],
                AF.Exp, scale=1.0 / _SIG,
            )

            def emit_sums(S, base, pT, b):
                # even image -> partitions 0:64, odd -> 64:128
                for j in (0, 1):
                    lsl = lhs_t[:, 0:64] if j == 0 else lhs_t[:, 64:128]
                    o = pT[0:64, b, 0:_NW] if j == 0 else pT[64:128, b, 0:_NW]
                    for k in range(4):
                        off = base + k * 2 * _NW + j * _NW
                        nc.tensor.matmul(
                            o, lsl, S[:, off:off + _NW],
                            start=(k == 0), stop=(k == 3),
                            skip_group_check=True,
                        )

            # ---- M = E*X (fills DVE while ACT runs the F exp)
            MX = mx.tile([128, 4 * _FD], bf16, tag="MX")
            nc.vector.tensor_tensor(
                MX[:, 0:2 * _FD], EF[:, 0:2 * _FD], Xf, op=MUL
            )
            emit_sums(EF, 0, pEF, 0)
            emit_sums(MX, 0, pMX, 0)

            # ---- combine of the PREVIOUS pair (deferred: more DVE filler
            # between DICEU and the F-dependent FX multiply)
            if prev is not None:
                emit_combine(*prev)

            # ---- FX = F*X, then F/X sums
            nc.vector.tensor_tensor(
                MX[:, 2 * _FD:4 * _FD], EF[:, 2 * _FD:4 * _FD], Xf, op=MUL
            )
            emit_sums(EF, 2 * _FD, pEF, 1)
            emit_sums(MX, 2 * _FD, pMX, 1)
            prev = (p, pEF, pMX)
        emit_combine(*prev)

    nc.finalize()
    _cache["nc"] = nc
    return nc


def _lhs_const():
    import ml_dtypes

    lhs = np.zeros((128, 128), np.float32)
    for q in range(64):
        lhs[q, q] = 0.125
        lhs[q, 64 + q] = 0.125
        lhs[64 + q, q] = 0.125
        lhs[64 + q, 64 + q] = 0.125
    return lhs.astype(ml_dtypes.bfloat16), np.eye(128, dtype=np.float32).astype(
        ml_dtypes.bfloat16
    )


def _prep_x(x, n):
    import ml_dtypes

    # [img, od,kd, ohp,oh3,kh, ow,kw] -> [img, (kd,od,oh3), (kh,kw,ohp,ow)]
    return np.ascontiguousarray(
        (x * _SIG)
        .reshape(n, 8, 2, _OHP, _OH3, 2, 56, 2)
        .transpose(0, 2, 1, 4, 5, 7, 3, 6)
        .reshape(n, 128, _FD)
        .astype(ml_dtypes.bfloat16)
    )


def _prep_beta(beta):
    # beta [8,56,56] -> [q=(od,oh3), (ohp,ow)], dup to 128 partitions
    bq = (
        beta.reshape(_OD, _OHP, _OH3, _OW)
        .transpose(0, 2, 1, 3)
        .reshape(64, _NW)
        .astype(np.float32)
    )
    ibe = _SIG / np.maximum(1.0 - bq, 1e-7)
    ibf = _SIG / np.maximum(bq, 1e-7)
    return (
        np.ascontiguousarray(np.concatenate([ibe, ibe], axis=0)),
        np.ascontiguousarray(np.concatenate([ibf, ibf], axis=0)),
    )


def _unprep_out(outs, B, C):
    # outs [cores, PAIRS, 128, 392] -> [B, C, OD, OH, OW]
    n = B * C
    return np.ascontiguousarray(
        outs.reshape(n, _OD, _OH3, _OHP, _OW)
        .transpose(0, 1, 3, 2, 4)
        .reshape(B, C, _OD, _OH, _OW)
    )


def _pairify(x_core):
    # [32, 128, (khw4, 392)] -> [16, 128, (khw4, img2, 392)]
    return np.ascontiguousarray(
        x_core.reshape(_PAIRS, 2, 128, 4, _NW)
        .transpose(0, 2, 3, 1, 4)
        .reshape(_PAIRS, 128, 2 * _FD)
    )


def kernel(**inputs):
    x = np.asarray(inputs["x"], dtype=np.float32)
    beta = np.asarray(inputs["beta"], dtype=np.float32)
    B, C = x.shape[0], x.shape[1]
    n = B * C
    x_r = _prep_x(x, n)
    ibe, ibf = _prep_beta(beta)
    nc = _build()
    lhs, idn = _lhs_const()
    in_maps = [
        {
            "x": _pairify(x_r[i * _IMGS:(i + 1) * _IMGS]),
            "lhs": lhs,
            "idn": idn,
            "invbe": ibe,
            "invbf": ibf,
        }
        for i in range(_NCORES)
    ]
    from concourse.bass_utils import run_bass_kernel_spmd

    res = run_bass_kernel_spmd(nc, in_maps, core_ids=list(range(_NCORES)))
    outs = np.stack(
        [np.asarray(res.results[i]["out"]).astype(np.float32) for i in range(_NCORES)]
    )
    return _unprep_out(outs, B, C)


if __name__ == "__main__":
    _build()
    print("build OK")


# revision 42
# speedup vs baseline: 1.0256x; 1.0256x over previous
"""AdaPool3d Trainium2 kernel — 8-core data parallel, v2.

x [4,64,16,112,112] f32, beta [8,56,56] f32 -> out [4,64,8,56,56] f32.
256 (b,c) images sharded 32/core, processed as 16 pairs/core.

Per image [16,112,112], SBUF layout X [128,1568] bf16 (DMA-cast of
SIG*x): partition p=(kd,od,oh3), free f=(kh,kw,ohp,ow); d=2*od+kd,
h=16*ohp+2*oh3+kh, w=2*ow+kw.  All window reductions are PE matmuls
against one constant lhsT (0.125 kd-pair selector, kd-dup for the avg,
col-halves 0:64/64:128 for even/odd image of a pair so the four
e/ex/f/fx sums of a pair share PSUM banks as [128,392]).

Math: u = x/avg via RECIP1 (NOT-trick+1NR) on the small [128,392] avg;
DS = DICEU(X, rA) = u*nb*(C1 - (1+u^2)*nb), nb=NOT(1+u^2) — an 8-op
fused DVE op whose NR seed scale is absorbed into C1=-8.5, making
DS = dsc/2 * G with G=18.0329.  The host pre-scales x by SIG=G/2 so a
single ACT exp over [X|DS] with scale=1/SIG yields [e^x | e^dsc].
em/edscw divisions fold beta and all scales into two host constants
invBE = SIG/(1-beta), invBF = SIG/beta; out = pM/(pE*invBE) +
pX/(pF*invBF) via two DIV1 ops and one bf16 add per pair.
"""

import numpy as np

_NCORES = 8
_IMGS = 32          # images per core
_PAIRS = 16
_OD, _OH, _OW = 8, 56, 56
_OHP, _OH3 = 7, 8   # oh = 8*ohp + oh3
_FD = 1568          # per-partition free elems = 4*392
_NW = 392           # windows per q-group = 7*56

_G = 18.032925      # DICEU gain (C1 = -8.5)
_SIG = _G / 2.0
_C1D = -8.5
_R0, _R1 = -0.23549792, 2.0017324  # NOT-trick + 1NR recip constants

_cache = {}


def _register_op(name, spec):
    from concourse.dve_spec import lower, _has_src1
    from concourse import dve_ops
    from concourse.dve_uop import DveOpSpec

    for op in dve_ops.OPS:
        if op.name == name:
            return op
    row = dve_ops._CUSTOM_DVE_ROW_BASE + len(dve_ops.OPS)
    assert row < 0x20
    dve_ops._SUB_OPCODE_FOR_NAME[name] = row
    shas = {}
    for ver in ("v3", "v4"):
        try:
            uops = lower(spec, ver=ver)
            shas[ver] = DveOpSpec(
                name=name, opcode=row, uops=uops, rd1_en=_has_src1(spec)
            ).sha(ver)
        except Exception:
            pass
    op = dve_ops.DveOp(name, spec, subdim=False, uops_sha=shas)
    dve_ops.OPS.append(op)
    dve_ops.CUSTOM_DVE_SPECS[name] = spec
    return op


def _np_not(z):
    return (~np.asarray(z, np.float32).view(np.int32)).view(np.float32)


def _np_r1(z, s0, s1):
    y0 = _np_not(z) * np.float32(s0)
    return y0 * (np.float32(s1) - z * y0)


def _register_custom_ops():
    """RECIP1_EPS: approx(1/(x+eps)), 1-NR; DICEU: fused u=x*r,
    dsc-like = u*nb*(C1-(1+u^2)*nb); DIV1: num*approx(1/den), 1-NR."""
    from concourse.dve_spec import Spec, Src0, Src1, Bin, AluOp, sq, One, C0, C1, C2

    _ze = Src0 + C2
    _nbe = Bin(AluOp.BITWISE_NOT, _ze, _ze)
    _y0e = _nbe * C0
    _y1e = _y0e * (C1 - _ze * _y0e)
    recip1 = _register_op(
        "RECIP1_EPS_ANT",
        Spec(
            body=_y1e,
            reference=lambda in0, in1, s0, s1, imm2: _np_r1(
                in0.astype(np.float32) + np.float32(imm2), s0, s1
            ),
        ),
    )

    _u = Src0 * Src1
    _z = sq(_u) + One
    _nb = Bin(AluOp.BITWISE_NOT, _z, _z)
    _q = _z * _nb
    _w = C1 - _q
    _m = _u * _nb

    def _diceu_ref(in0, in1, s0, s1, imm2):
        u = in0.astype(np.float32) * in1.astype(np.float32)
        z = (1.0 + u * u).astype(np.float32)
        nb = _np_not(z)
        return (u * nb).astype(np.float32) * (
            np.float32(s1) - (z * nb).astype(np.float32)
        )

    diceu = _register_op("DICEU_ANT", Spec(body=_m * _w, reference=_diceu_ref))

    _nbd = Bin(AluOp.BITWISE_NOT, Src1, Src1)
    _y0d = _nbd * C0
    _y1d = _y0d * (C1 - Src1 * _y0d)
    div1 = _register_op(
        "DIV1_ANT",
        Spec(
            body=Src0 * _y1d,
            reference=lambda in0, in1, s0, s1, imm2: in0
            * _np_r1(in1.astype(np.float32), s0, s1),
        ),
    )
    return recip1, diceu, div1


def _build():
    if "nc" in _cache:
        return _cache["nc"]
    import concourse.bass as bass
    import concourse.bacc as bacc
    import concourse.mybir as mybir
    from concourse.tile import TileContext
    from contextlib import ExitStack

    RECIP1, DICEU, DIV1 = _register_custom_ops()
    f32, bf16 = mybir.dt.float32, mybir.dt.bfloat16
    AF = mybir.ActivationFunctionType
    MUL, ADD = mybir.AluOpType.mult, mybir.AluOpType.add

    nc = bacc.Bacc(None, target_bir_lowering=False, debug=False)
    # host pre-rearranged pairs: x [pair, p=(kd,od,oh3), f=(kh,kw,img,ohp,ow)]
    x_d = nc.dram_tensor("x", [_PAIRS, 128, 2 * _FD], bf16, kind="ExternalInput")
    lhs_d = nc.dram_tensor("lhs", [128, 128], bf16, kind="ExternalInput")
    idn_d = nc.dram_tensor("idn", [128, 128], bf16, kind="ExternalInput")
    ibe_d = nc.dram_tensor("invbe", [128, _NW], f32, kind="ExternalInput")
    ibf_d = nc.dram_tensor("invbf", [128, _NW], f32, kind="ExternalInput")
    out_d = nc.dram_tensor("out", [_PAIRS, 128, _NW], bf16, kind="ExternalOutput")

    x_ap = x_d.ap()
    out_ap = out_d.ap()

    with TileContext(nc) as tc, ExitStack() as ctx:
        const = ctx.enter_context(tc.tile_pool(name="const", bufs=1))
        xin = ctx.enter_context(tc.tile_pool(name="xin", bufs=4))
        ef = ctx.enter_context(tc.tile_pool(name="ef", bufs=4))
        mx = ctx.enter_context(tc.tile_pool(name="mx", bufs=4))
        sm = ctx.enter_context(tc.tile_pool(name="sm", bufs=3))
        pss = ctx.enter_context(tc.tile_pool(name="pss", bufs=2, space="PSUM"))

        # consts on the gpsimd queue so pair-0's x DMA leads the sync queue
        lhs_t = const.tile([128, 128], bf16, name="lhsT")
        nc.gpsimd.dma_start(out=lhs_t[:], in_=lhs_d.ap())
        idn_t = const.tile([128, 128], bf16, name="idnT")
        nc.gpsimd.dma_start(out=idn_t[:], in_=idn_d.ap())
        ibef_t = const.tile([128, 2 * _NW], f32, name="ibef")
        nc.gpsimd.dma_start(out=ibef_t[:, 0:_NW], in_=ibe_d.ap())
        nc.gpsimd.dma_start(out=ibef_t[:, _NW:2 * _NW], in_=ibf_d.ap())

        def emit_combine(pp, pEF, pMX):
            # ---- combine (per pair, [128, 2*392] = 2 images x {em, ed})
            cEF = sm.tile([128, 2 * _NW], f32, tag="cEF", name="cEF")
            nc.vector.tensor_tensor(
                cEF[:], pEF[:, :, 0:_NW], ibef_t[:], op=MUL
            )
            emed = sm.tile([128, 2 * _NW], bf16, tag="emed", name="emed")
            nc.vector._custom_dve(
                DIV1, out=emed[:], in0=pMX[:, :, 0:_NW], in1=cEF[:],
                s0=_R0, s1=_R1,
            )
            # em + ed on the (idle) PE: two identity matmuls accumulating
            # into the pMX bank the DIV1 just freed; ACT evacuates w/ cast.
            po = pMX[:, 0, 0:_NW]
            nc.tensor.matmul(po, idn_t[:], emed[:, 0:_NW],
                             start=True, stop=False, skip_group_check=True)
            nc.tensor.matmul(po, idn_t[:], emed[:, _NW:2 * _NW],
                             start=False, stop=True, skip_group_check=True)
            oc = sm.tile([128, _NW], bf16, tag="oc", name="oc")
            nc.scalar.copy(oc[:], po)
            nc.gpsimd.dma_start(out=out_ap[pp], in_=oc[:])

        prev = None
        for p in range(_PAIRS):
            # ---- load pair; [Xpair | DSpair] tile, f=(kh,kw,img,ohp,ow)
            XX = xin.tile([128, 4 * _FD], bf16, tag="XX")
            nc.sync.dma_start(out=XX[:, 0:2 * _FD], in_=x_ap[p])
            Xf = XX[:, 0:2 * _FD]

            # ---- PSUM: pEF holds avg (kd-dup) then E/F sums; pMX holds M/X
            pEF = pss.tile([128, 2, 512], f32, tag="pEF", name="pEF")
            pMX = pss.tile([128, 2, 512], f32, tag="pMX", name="pMX")
            for j in (0, 1):
                for k in range(4):
                    nc.tensor.matmul(
                        pEF[:, j, 0:_NW], lhs_t[:],
                        XX[:, k * 2 * _NW + j * _NW:k * 2 * _NW + (j + 1) * _NW],
                        start=(k == 0), stop=(k == 3),
                        skip_group_check=True,
                    )

            # ---- E = exp(x) early: depends only on the DMA, not on DICEU
            EF = ef.tile([128, 4 * _FD], bf16, tag="EF")
            nc.scalar.activation(EF[:, 0:2 * _FD], Xf, AF.Exp, scale=1.0 / _SIG)

            # ---- rA2 = approx 1/avg  (f32, [128, 784])
            rA2 = sm.tile([128, 2 * _NW], f32, tag="rA2")
            nc.vector._custom_dve(
                RECIP1, out=rA2[:], in0=pEF[:, :, 0:_NW],
                s0=_R0, s1=_R1, imm2=1e-12,
            )

            # ---- DS = DICEU(X, bcast rA2) into XX[:, 2FD:4FD]
            rA_b = (
                rA2[:]
                .rearrange("q (s n) -> q s n", s=1)
                .broadcast_to([128, 4, 2 * _NW])
            )
            nc.vector._custom_dve(
                DICEU, out=XX[:, 2 * _FD:4 * _FD], in0=Xf, in1=rA_b, s1=_C1D
            )

            # ---- F = exp(dsc)
            nc.scalar.activation(
                EF[:, 2 * _FD:4 * _FD], XX[:, 2 * _FD:4 * _FD],
                AF.Exp, scale=1.0 / _SIG,
            )

            def emit_sums(S, base, pT, b):
                # even image -> partitions 0:64, odd -> 64:128
                for j in (0, 1):
                    lsl = lhs_t[:, 0:64] if j == 0 else lhs_t[:, 64:128]
                    o = pT[0:64, b, 0:_NW] if j == 0 else pT[64:128, b, 0:_NW]
                    for k in range(4):
                        off = base + k * 2 * _NW + j * _NW
                        nc.tensor.matmul(
                            o, lsl, S[:, off:off + _NW],
                            start=(k == 0), stop=(k == 3),
                            skip_group_check=True,
                        )

            # ---- M = E*X (fills DVE while ACT runs the F exp)
            MX = mx.tile([128, 4 * _FD], bf16, tag="MX")
            nc.vector.tensor_tensor(
                MX[:, 0:2 * _FD], EF[:, 0:2 * _FD], Xf, op=MUL
            )
            emit_sums(EF, 0, pEF, 0)
            emit_sums(MX, 0, pMX, 0)

            # ---- combine of the PREVIOUS pair (deferred: more DVE filler
            # between DICEU and the F-dependent FX multiply)
            if prev is not None:
                emit_combine(*prev)

            # ---- FX = F*X, then F/X sums
            nc.vector.tensor_tensor(
                MX[:, 2 * _FD:4 * _FD], EF[:, 2 * _FD:4 * _FD], Xf, op=MUL
            )
            emit_sums(EF, 2 * _FD, pEF, 1)
            emit_sums(MX, 2 * _FD, pMX, 1)
            prev = (p, pEF, pMX)
        emit_combine(*prev)

    nc.finalize()
    _cache["nc"] = nc
    return nc


def _lhs_const():
    import ml_dtypes

    lhs = np.zeros((128, 128), np.float32)
    for q in range(64):
        lhs[q, q] = 0.125
        lhs[q, 64 + q] = 0.125
        lhs[64 + q, q] = 0.125
        lhs[64 + q, 64 + q] = 0.125
    return lhs.astype(ml_dtypes.bfloat16), np.eye(128, dtype=np.float32).astype(
        ml_dtypes.bfloat16
    )


def _prep_x(x, n):
    import ml_dtypes

    # [img, od,kd, ohp,oh3,kh, ow,kw] -> [img, (kd,od,oh3), (kh,kw,ohp,ow)]
    return np.ascontiguousarray(
        (x * _SIG)
        .reshape(n, 8, 2, _OHP, _OH3, 2, 56, 2)
        .transpose(0, 2, 1, 4, 5, 7, 3, 6)
        .reshape(n, 128, _FD)
        .astype(ml_dtypes.bfloat16)
    )


def _prep_beta(beta):
    # beta [8,56,56] -> [q=(od,oh3), (ohp,ow)], dup to 128 partitions
    bq = (
        beta.reshape(_OD, _OHP, _OH3, _OW)
        .transpose(0, 2, 1, 3)
        .reshape(64, _NW)
        .astype(np.float32)
    )
    ibe = _SIG / np.maximum(1.0 - bq, 1e-7)
    ibf = _SIG / np.maximum(bq, 1e-7)
    return (
        np.ascontiguousarray(np.concatenate([ibe, ibe], axis=0)),
        np.ascontiguousarray(np.concatenate([ibf, ibf], axis=0)),
    )


def _unprep_out(outs, B, C):
    # outs [cores, PAIRS, 128, 392] -> [B, C, OD, OH, OW]
    n = B * C
    return np.ascontiguousarray(
        outs.reshape(n, _OD, _OH3, _OHP, _OW)
        .transpose(0, 1, 3, 2, 4)
        .reshape(B, C, _OD, _OH, _OW)
    )


def _pairify(x_core):
    # [32, 128, (khw4, 392)] -> [16, 128, (khw4, img2, 392)]
    return np.ascontiguousarray(
        x_core.reshape(_PAIRS, 2, 128, 4, _NW)
        .transpose(0, 2, 3, 1, 4)
        .reshape(_PAIRS, 128, 2 * _FD)
    )


def kernel(**inputs):
    x = np.asarray(inputs["x"], dtype=np.float32)
    beta = np.asarray(inputs["beta"], dtype=np.float32)
    B, C = x.shape[0], x.shape[1]
    n = B * C
    x_r = _prep_x(x, n)
    ibe, ibf = _prep_beta(beta)
    nc = _build()
    lhs, idn = _lhs_const()
    in_maps = [
        {
            "x": _pairify(x_r[i * _IMGS:(i + 1) * _IMGS]),
            "lhs": lhs,
            "idn": idn,
            "invbe": ibe,
            "invbf": ibf,
        }
        for i in range(_NCORES)
    ]
    from concourse.bass_utils import run_bass_kernel_spmd

    res = run_bass_kernel_spmd(nc, in_maps, core_ids=list(range(_NCORES)))
    outs = np.stack(
        [np.asarray(res.results[i]["out"]).astype(np.float32) for i in range(_NCORES)]
    )
    return _unprep_out(outs, B, C)


if __name__ == "__main__":
    _build()
    print("build OK")
